# revision 3
# baseline (speedup 1.0000x reference)
"""Contrastive-loss kernel for Trainium2 (8 NeuronCores, SPMD).

The reference builds NxN pairwise matrices, but every term collapses to a
closed form over five O(N) reductions of p = sigmoid(y_pred) and t = y_true:

    S1 = sum p          S2 = sum p^2
    Spt = sum p*t       Sp2t = sum p^2*t      St = sum t

    sum_dist_sq = 2*N*S2 - 2*S1^2
    mean(loss_diff) = sum_dist_sq * 2*n_pos*n_neg / N^2
    ss_pos + ss_neg = (Sp2t - Spt^2/n_pos) + ((S2-Sp2t) - (S1-Spt)^2/n_neg)
    mean(loss_same) = (ss_pos+ss_neg) * (n_pos^2+n_neg^2) / N^2

Each of the 8 cores reduces a 1024-element shard (as a [128, 8] tile) and
emits [128, 5] per-partition partials; the host sums partials in float64 and
applies the closed form.
"""

import numpy as np

N = 8192
N_CORES = 8
SHARD = N // N_CORES  # 1024
P = 128
F = SHARD // P  # 8

_NC = None  # compiled Bass program, built once


def _build_bass():
    import concourse.bass as bass
    import concourse.mybir as mybir

    nc = bass.Bass()
    f32 = mybir.dt.float32

    x_d = nc.dram_tensor("x", [P, F], f32, kind="ExternalInput")
    t_d = nc.dram_tensor("t", [P, F], f32, kind="ExternalInput")
    out_d = nc.dram_tensor("partials", [P, 5], f32, kind="ExternalOutput")

    AF = mybir.ActivationFunctionType
    ALU = mybir.AluOpType

    with (
        nc.sbuf_tensor([P, F], f32) as xa,
        nc.sbuf_tensor([P, F], f32) as tf,
        nc.sbuf_tensor([P, F], f32) as p,
        nc.sbuf_tensor([P, F], f32) as tcopy,
        nc.sbuf_tensor([P, F], f32) as p2,
        nc.sbuf_tensor([P, F], f32) as pt,
        nc.sbuf_tensor([P, F], f32) as p2t,
        nc.sbuf_tensor([P, 5], f32) as acc,
        nc.semaphore("dma_in") as dma_in,
        nc.semaphore("act_done") as act_done,
        nc.semaphore("dve_done") as dve_done,
        nc.Block() as block,
    ):

        @block.sync
        def _(sync):
            sync.dma_start(xa[:], x_d[:]).then_inc(dma_in, 16)
            sync.dma_start(tf[:], t_d[:]).then_inc(dma_in, 16)
            sync.wait_ge(act_done, 2)
            sync.wait_ge(dve_done, 3)
            sync.dma_start(out_d[:], acc[:]).then_inc(dma_in, 16)
            sync.wait_ge(dma_in, 48)

        @block.scalar
        def _(scalar):
            scalar.wait_ge(dma_in, 32)
            # p = sigmoid(x); acc[:,0] = rowsum(p)
            scalar.activation(
                p[:], xa[:], AF.Sigmoid, accum_out=acc[:, 0:1]
            ).then_inc(act_done, 1)
            # acc[:,4] = rowsum(t)
            scalar.activation(
                tcopy[:], tf[:], AF.Copy, accum_out=acc[:, 4:5]
            ).then_inc(act_done, 1)

        @block.vector
        def _(vector):
            vector.wait_ge(act_done, 1)
            vector.wait_ge(dma_in, 32)
            # p2 = (p*1)*p; acc[:,1] = rowsum(p2)
            vector.scalar_tensor_tensor(
                out=p2[:], in0=p[:], scalar=1.0, in1=p[:],
                op0=ALU.mult, op1=ALU.mult, accum_out=acc[:, 1:2],
            ).then_inc(dve_done, 1)
            # pt = (p*1)*t; acc[:,2] = rowsum(pt)
            vector.scalar_tensor_tensor(
                out=pt[:], in0=p[:], scalar=1.0, in1=tf[:],
                op0=ALU.mult, op1=ALU.mult, accum_out=acc[:, 2:3],
            ).then_inc(dve_done, 1)
            # p2t = (p2*1)*t; acc[:,3] = rowsum(p2t) — wait for the p2 write
            # to retire (same-engine RAW is not interlocked)
            vector.wait_ge(dve_done, 1)
            vector.scalar_tensor_tensor(
                out=p2t[:], in0=p2[:], scalar=1.0, in1=tf[:],
                op0=ALU.mult, op1=ALU.mult, accum_out=acc[:, 3:4],
            ).then_inc(dve_done, 1)

    return nc


def _get_nc():
    global _NC
    if _NC is None:
        _NC = _build_bass()
    return _NC


def _make_in_maps(y_pred, y_true):
    x = np.ascontiguousarray(np.asarray(y_pred, dtype=np.float32).reshape(-1))
    t = np.asarray(y_true).astype(np.float32).reshape(-1)
    in_maps = []
    for c in range(N_CORES):
        sl = slice(c * SHARD, (c + 1) * SHARD)
        in_maps.append(
            {
                "x": x[sl].reshape(P, F),
                "t": np.ascontiguousarray(t[sl]).reshape(P, F),
            }
        )
    return in_maps


def _combine(partials_list):
    # partials_list: per-core [P, 5] float32 arrays
    S = np.zeros(5, dtype=np.float64)
    for part in partials_list:
        S += part.astype(np.float64).sum(axis=0)
    S1, S2, Spt, Sp2t, St = S
    n = float(N)
    n_pos = St
    n_neg = n - St
    sum_dist_sq = 2.0 * n * S2 - 2.0 * S1 * S1
    ss_pos = Sp2t - Spt * Spt / n_pos
    Sn = S1 - Spt
    Sn2 = S2 - Sp2t
    ss_neg = Sn2 - Sn * Sn / n_neg
    loss = (
        sum_dist_sq * (2.0 * n_pos * n_neg) / (n * n)
        + (ss_pos + ss_neg) * (n_pos * n_pos + n_neg * n_neg) / (n * n)
    )
    return np.asarray(loss, dtype=np.float32)


def kernel(y_pred, y_true, epoch=None, **_unused):
    from concourse.bass_utils import run_bass_kernel_spmd

    nc = _get_nc()
    in_maps = _make_in_maps(y_pred, y_true)
    res = run_bass_kernel_spmd(nc, in_maps, list(range(N_CORES)))
    partials = [r["partials"] for r in res.results]
    return _combine(partials)


# revision 11
# speedup vs baseline: 1.1163x; 1.1163x over previous
"""Contrastive-loss kernel for Trainium2 (8 NeuronCores, SPMD).

The reference builds NxN pairwise matrices, but every term collapses to a
closed form over five O(N) reductions of p = sigmoid(y_pred) and t = y_true:

    S1 = sum p          S2 = sum p^2
    Spt = sum p*t       Sp2t = sum p^2*t      St = sum t

    sum_dist_sq = 2*N*S2 - 2*S1^2
    mean(loss_diff) = sum_dist_sq * 2*n_pos*n_neg / N^2
    ss_pos + ss_neg = (Sp2t - Spt^2/n_pos) + ((S2-Sp2t) - (S1-Spt)^2/n_neg)
    mean(loss_same) = (ss_pos+ss_neg) * (n_pos^2+n_neg^2) / N^2

Each of the 8 cores reduces a 1024-element shard (x and t packed as one
[128, 16] tile so the input lands in a single DMA) and emits [128, 5]
per-partition partials; the host sums partials in float64 and applies the
closed form.

Device-side structure per core (raw Bass, manual semaphores):
  sync  : DMA xt in -> (wait compute) -> DMA partials out (completion is
          covered by the block-exit DRAIN, no extra sem round-trip)
  scalar: prime Sigmoid PWP table on a const AP before the DMA wait (the
          ~1.3us table load overlaps the input DMA), then
          Sigmoid(x)+rowsum(p), Copy(t)+rowsum(t)
  vector: three scalar_tensor_tensor ops with fused row-sum accumulators:
          p^2, p*t, p^2*t
"""

import numpy as np

N = 8192
N_CORES = 8
SHARD = N // N_CORES  # 1024
P = 128
F = SHARD // P  # 8

_NC = None  # compiled Bass program, built once


def _build_bass(variant="v2"):
    import concourse.bass as bass
    import concourse.mybir as mybir

    nc = bass.Bass()
    f32 = mybir.dt.float32

    xt_d = nc.dram_tensor("xt", [P, 2 * F], f32, kind="ExternalInput")
    out_d = nc.dram_tensor("partials", [P, 5], f32, kind="ExternalOutput")

    AF = mybir.ActivationFunctionType
    ALU = mybir.AluOpType

    with (
        nc.sbuf_tensor([P, 2 * F], f32) as xt,
        nc.sbuf_tensor([P, 1], f32) as warm,
        nc.sbuf_tensor([P, F], f32) as p,
        nc.sbuf_tensor([P, F], f32) as tcopy,
        nc.sbuf_tensor([P, F], f32) as p2,
        nc.sbuf_tensor([P, F], f32) as pt,
        nc.sbuf_tensor([P, F], f32) as p2t,
        nc.sbuf_tensor([P, 5], f32) as acc,
        nc.semaphore("dma_in") as dma_in,
        nc.semaphore("dma_in_g") as dma_in_g,
        nc.semaphore("act_done") as act_done,
        nc.semaphore("dve_done") as dve_done,
        nc.Block() as block,
    ):
        xa = xt[:, 0:F]
        tf = xt[:, F : 2 * F]
        const0 = nc.const_aps.tensor(0.0, (P, 1), f32)

        dma_engine = "gpsimd" if variant == "v2g" else "sync"

        in_sem = dma_in_g if dma_engine == "gpsimd" else dma_in

        def dma_prog(eng):
            eng.dma_start(xt[:], xt_d[:]).then_inc(in_sem, 16)

        if dma_engine == "sync":

            @block.sync
            def _(sync):
                dma_prog(sync)
                sync.wait_ge(act_done, 2)
                sync.wait_ge(dve_done, 3)
                sync.dma_start(out_d[:], acc[:]).then_inc(dma_in, 16)
        else:

            @block.gpsimd
            def _(gpsimd):
                dma_prog(gpsimd)

            @block.sync
            def _(sync):
                sync.wait_ge(act_done, 2)
                sync.wait_ge(dve_done, 3)
                sync.dma_start(out_d[:], acc[:]).then_inc(dma_in, 16)

        @block.scalar
        def _(scalar):
            # Prime the Sigmoid PWP table before the data arrives.
            scalar.activation(warm[:], const0, AF.Sigmoid)
            scalar.wait_ge(in_sem, 16)
            # p = sigmoid(x); acc[:,0] = rowsum(p)
            scalar.activation(
                p[:], xa, AF.Sigmoid, accum_out=acc[:, 0:1]
            ).then_inc(act_done, 1)
            # acc[:,4] = rowsum(t)
            scalar.activation(
                tcopy[:], tf, AF.Copy, accum_out=acc[:, 4:5]
            ).then_inc(act_done, 1)

        @block.vector
        def _(vector):
            vector.wait_ge(act_done, 1)
            # p2 = (p*1)*p; acc[:,1] = rowsum(p2)
            vector.scalar_tensor_tensor(
                out=p2[:], in0=p[:], scalar=1.0, in1=p[:],
                op0=ALU.mult, op1=ALU.mult, accum_out=acc[:, 1:2],
            ).then_inc(dve_done, 1)
            # pt = (p*1)*t; acc[:,2] = rowsum(pt)
            vector.scalar_tensor_tensor(
                out=pt[:], in0=p[:], scalar=1.0, in1=tf,
                op0=ALU.mult, op1=ALU.mult, accum_out=acc[:, 2:3],
            ).then_inc(dve_done, 1)
            # p2t = (p2*1)*t; acc[:,3] = rowsum(p2t) — wait for the p2 write
            # to retire (same-engine RAW is not interlocked)
            vector.wait_ge(dve_done, 1)
            vector.scalar_tensor_tensor(
                out=p2t[:], in0=p2[:], scalar=1.0, in1=tf,
                op0=ALU.mult, op1=ALU.mult, accum_out=acc[:, 3:4],
            ).then_inc(dve_done, 1)

    return nc


def _build_floor():
    """Minimal kernel: one tiny output DMA — measures the NEFF protocol floor."""
    import concourse.bass as bass
    import concourse.mybir as mybir

    nc = bass.Bass()
    f32 = mybir.dt.float32
    out_d = nc.dram_tensor("partials", [P, 1], f32, kind="ExternalOutput")
    with nc.Block() as block:
        const0 = nc.const_aps.tensor(0.0, (P, 1), f32)

        @block.sync
        def _(sync):
            with nc.semaphore("floor_sem") as fs:
                sync.dma_start(out_d[:], const0).then_inc(fs, 16)

    return nc


def _get_nc():
    global _NC
    if _NC is None:
        _NC = _build_bass()
    return _NC


def _make_in_maps(y_pred, y_true):
    x = np.asarray(y_pred, dtype=np.float32).reshape(-1)
    t = np.asarray(y_true).astype(np.float32).reshape(-1)
    in_maps = []
    for c in range(N_CORES):
        sl = slice(c * SHARD, (c + 1) * SHARD)
        xt = np.concatenate(
            [x[sl].reshape(P, F), t[sl].reshape(P, F)], axis=1
        )
        in_maps.append({"xt": np.ascontiguousarray(xt)})
    return in_maps


def _combine(partials_list):
    # partials_list: per-core [P, 5] float32 arrays
    S = np.zeros(5, dtype=np.float64)
    for part in partials_list:
        S += part.astype(np.float64).sum(axis=0)
    S1, S2, Spt, Sp2t, St = S
    n = float(N)
    n_pos = St
    n_neg = n - St
    sum_dist_sq = 2.0 * n * S2 - 2.0 * S1 * S1
    ss_pos = Sp2t - Spt * Spt / n_pos
    Sn = S1 - Spt
    Sn2 = S2 - Sp2t
    ss_neg = Sn2 - Sn * Sn / n_neg
    loss = (
        sum_dist_sq * (2.0 * n_pos * n_neg) / (n * n)
        + (ss_pos + ss_neg) * (n_pos * n_pos + n_neg * n_neg) / (n * n)
    )
    return np.asarray(loss, dtype=np.float32)


def kernel(y_pred, y_true, epoch=None, **_unused):
    from concourse.bass_utils import run_bass_kernel_spmd

    nc = _get_nc()
    in_maps = _make_in_maps(y_pred, y_true)
    res = run_bass_kernel_spmd(nc, in_maps, list(range(N_CORES)))
    partials = [r["partials"] for r in res.results]
    return _combine(partials)


# revision 13
# speedup vs baseline: 1.2869x; 1.1527x over previous
"""Contrastive-loss kernel for Trainium2 (8 NeuronCores, SPMD).

The reference builds NxN pairwise matrices, but every term collapses to a
closed form over five O(N) reductions of p = sigmoid(y_pred) and t = y_true:

    S1 = sum p          S2 = sum p^2
    Spt = sum p*t       Sp2t = sum p^2*t      St = sum t

    sum_dist_sq = 2*N*S2 - 2*S1^2
    mean(loss_diff) = sum_dist_sq * 2*n_pos*n_neg / N^2
    ss_pos + ss_neg = (Sp2t - Spt^2/n_pos) + ((S2-Sp2t) - (S1-Spt)^2/n_neg)
    mean(loss_same) = (ss_pos+ss_neg) * (n_pos^2+n_neg^2) / N^2

Each of the 8 cores reduces a 1024-element shard (x and t packed as one
[128, 16] tile so the input lands in a single DMA) and emits [128, 5]
per-partition partials; the host sums partials in float64 and applies the
closed form.

Device-side structure per core (raw Bass, manual semaphores):
  sync  : DMA xt in -> (wait compute) -> DMA partials out (completion is
          covered by the block-exit DRAIN, no extra sem round-trip)
  scalar: prime Sigmoid PWP table on a const AP before the DMA wait (the
          ~1.3us table load overlaps the input DMA), then
          Sigmoid(x)+rowsum(p), Copy(t)+rowsum(t)
  vector: three scalar_tensor_tensor ops with fused row-sum accumulators:
          p^2, p*t, p^2*t
"""

import numpy as np

N = 8192
N_CORES = 8
SHARD = N // N_CORES  # 1024
P = 128
F = SHARD // P  # 8

_NC = None  # compiled Bass program, built once


def _build_bass(variant="v2"):
    import concourse.bass as bass
    import concourse.mybir as mybir

    nc = bass.Bass()
    f32 = mybir.dt.float32

    xt_d = nc.dram_tensor("xt", [P, 2 * F], f32, kind="ExternalInput")
    out_d = nc.dram_tensor("partials", [P, 5], f32, kind="ExternalOutput")

    AF = mybir.ActivationFunctionType
    ALU = mybir.AluOpType

    with (
        nc.sbuf_tensor([P, 2 * F], f32) as xt,
        nc.sbuf_tensor([P, 1], f32) as warm,
        nc.sbuf_tensor([P, F], f32) as p,
        nc.sbuf_tensor([P, F], f32) as tcopy,
        nc.sbuf_tensor([P, F], f32) as p2,
        nc.sbuf_tensor([P, F], f32) as pt,
        nc.sbuf_tensor([P, F], f32) as p2t,
        nc.sbuf_tensor([P, 5], f32) as acc,
        nc.semaphore("dma_in") as dma_in,
        nc.semaphore("dma_in_g") as dma_in_g,
        nc.semaphore("act_done") as act_done,
        nc.semaphore("dve_done") as dve_done,
        nc.Block() as block,
    ):
        xa = xt[:, 0:F]
        tf = xt[:, F : 2 * F]
        const0 = nc.const_aps.tensor(0.0, (P, 1), f32)

        dma_engine = "gpsimd" if variant == "v2g" else "sync"

        in_sem = dma_in_g if dma_engine == "gpsimd" else dma_in

        def dma_prog(eng):
            eng.dma_start(
                xt[:], xt_d[:], single_packet=(variant == "v2sp")
            ).then_inc(in_sem, 16)

        if dma_engine == "sync":

            @block.sync
            def _(sync):
                dma_prog(sync)
                sync.wait_ge(act_done, 2)
                sync.wait_ge(dve_done, 3)
                sync.dma_start(out_d[:], acc[:]).then_inc(dma_in, 16)
        else:

            @block.gpsimd
            def _(gpsimd):
                dma_prog(gpsimd)

            @block.sync
            def _(sync):
                sync.wait_ge(act_done, 2)
                sync.wait_ge(dve_done, 3)
                sync.dma_start(out_d[:], acc[:]).then_inc(dma_in, 16)

        @block.scalar
        def _(scalar):
            # Prime the Sigmoid PWP table before the data arrives.
            scalar.activation(warm[:], const0, AF.Sigmoid)
            scalar.wait_ge(in_sem, 16)
            # p = sigmoid(x); acc[:,0] = rowsum(p)
            scalar.activation(
                p[:], xa, AF.Sigmoid, accum_out=acc[:, 0:1]
            ).then_inc(act_done, 1)
            # acc[:,4] = rowsum(t)
            scalar.activation(
                tcopy[:], tf, AF.Copy, accum_out=acc[:, 4:5]
            ).then_inc(act_done, 1)

        @block.vector
        def _(vector):
            vector.wait_ge(act_done, 1)
            # p2 = (p*1)*p; acc[:,1] = rowsum(p2)
            vector.scalar_tensor_tensor(
                out=p2[:], in0=p[:], scalar=1.0, in1=p[:],
                op0=ALU.mult, op1=ALU.mult, accum_out=acc[:, 1:2],
            ).then_inc(dve_done, 1)
            # pt = (p*1)*t; acc[:,2] = rowsum(pt)
            vector.scalar_tensor_tensor(
                out=pt[:], in0=p[:], scalar=1.0, in1=tf,
                op0=ALU.mult, op1=ALU.mult, accum_out=acc[:, 2:3],
            ).then_inc(dve_done, 1)
            # p2t = (p2*1)*t; acc[:,3] = rowsum(p2t) — wait for the p2 write
            # to retire (same-engine RAW is not interlocked)
            vector.wait_ge(dve_done, 1)
            vector.scalar_tensor_tensor(
                out=p2t[:], in0=p2[:], scalar=1.0, in1=tf,
                op0=ALU.mult, op1=ALU.mult, accum_out=acc[:, 3:4],
            ).then_inc(dve_done, 1)

    return nc


def _build_floor():
    """Minimal kernel: one tiny output DMA — measures the NEFF protocol floor."""
    import concourse.bass as bass
    import concourse.mybir as mybir

    nc = bass.Bass()
    f32 = mybir.dt.float32
    out_d = nc.dram_tensor("partials", [P, 1], f32, kind="ExternalOutput")
    with nc.Block() as block:
        const0 = nc.const_aps.tensor(0.0, (P, 1), f32)

        @block.sync
        def _(sync):
            with nc.semaphore("floor_sem") as fs:
                sync.dma_start(out_d[:], const0).then_inc(fs, 16)

    return nc


def _get_nc():
    global _NC
    if _NC is None:
        _NC = _build_bass("v2sp")
    return _NC


def _make_in_maps(y_pred, y_true):
    x = np.asarray(y_pred, dtype=np.float32).reshape(-1)
    t = np.asarray(y_true).astype(np.float32).reshape(-1)
    in_maps = []
    for c in range(N_CORES):
        sl = slice(c * SHARD, (c + 1) * SHARD)
        xt = np.concatenate(
            [x[sl].reshape(P, F), t[sl].reshape(P, F)], axis=1
        )
        in_maps.append({"xt": np.ascontiguousarray(xt)})
    return in_maps


def _combine(partials_list):
    # partials_list: per-core [P, 5] float32 arrays
    S = np.zeros(5, dtype=np.float64)
    for part in partials_list:
        S += part.astype(np.float64).sum(axis=0)
    S1, S2, Spt, Sp2t, St = S
    n = float(N)
    n_pos = St
    n_neg = n - St
    sum_dist_sq = 2.0 * n * S2 - 2.0 * S1 * S1
    ss_pos = Sp2t - Spt * Spt / n_pos
    Sn = S1 - Spt
    Sn2 = S2 - Sp2t
    ss_neg = Sn2 - Sn * Sn / n_neg
    loss = (
        sum_dist_sq * (2.0 * n_pos * n_neg) / (n * n)
        + (ss_pos + ss_neg) * (n_pos * n_pos + n_neg * n_neg) / (n * n)
    )
    return np.asarray(loss, dtype=np.float32)


def kernel(y_pred, y_true, epoch=None, **_unused):
    from concourse.bass_utils import run_bass_kernel_spmd

    nc = _get_nc()
    in_maps = _make_in_maps(y_pred, y_true)
    res = run_bass_kernel_spmd(nc, in_maps, list(range(N_CORES)))
    partials = [r["partials"] for r in res.results]
    return _combine(partials)


# revision 20
# speedup vs baseline: 1.2897x; 1.0022x over previous
"""Contrastive-loss kernel for Trainium2 (8 NeuronCores, SPMD).

The reference builds NxN pairwise matrices, but every term collapses to a
closed form over five O(N) reductions of p = sigmoid(y_pred) and t = y_true:

    S1 = sum p          S2 = sum p^2
    Spt = sum p*t       Sp2t = sum p^2*t      St = sum t

    sum_dist_sq = 2*N*S2 - 2*S1^2
    mean(loss_diff) = sum_dist_sq * 2*n_pos*n_neg / N^2
    ss_pos + ss_neg = (Sp2t - Spt^2/n_pos) + ((S2-Sp2t) - (S1-Spt)^2/n_neg)
    mean(loss_same) = (ss_pos+ss_neg) * (n_pos^2+n_neg^2) / N^2

Each of the 8 cores reduces a 1024-element shard (x and t packed as one
[32, 64] tile so the input lands in a single DMA; 32 partitions measured
marginally faster than 128 — shorter output DMA and accumulator reads) and
emits [32, 5] per-partition partials; the host sums partials in float64 and
applies the closed form.

Device-side structure per core (raw Bass, manual semaphores):
  sync  : DMA xt in -> (wait compute) -> DMA partials out (completion is
          covered by the block-exit DRAIN, no extra sem round-trip)
  scalar: prime Sigmoid PWP table on a const AP before the DMA wait (the
          ~1.3us table load overlaps the input DMA), then
          Sigmoid(x)+rowsum(p), Copy(t)+rowsum(t)
  vector: three scalar_tensor_tensor ops with fused row-sum accumulators:
          p^2, p*t, p^2*t
"""

import numpy as np

N = 8192
N_CORES = 8
SHARD = N // N_CORES  # 1024
P = 128
F = SHARD // P  # 8

VARIANT = "v5"  # [32, 64] tiles, single-packet input DMA
VP = 32         # partitions used by the default variant
VF = SHARD // VP

_NC = None  # compiled Bass program, built once


def _build_bass(variant="v2"):
    import concourse.bass as bass
    import concourse.mybir as mybir

    nc = bass.Bass()
    f32 = mybir.dt.float32

    if variant == "v4":
        return _build_bass_v4(nc, bass, mybir)

    # v5: same structure as v2sp but [32, 64] tiles — fewer partitions means
    # fewer DMA descriptor rows and shorter accumulator reads.
    PP = 32 if variant == "v5" else P
    FF = SHARD // PP

    xt_d = nc.dram_tensor("xt", [PP, 2 * FF], f32, kind="ExternalInput")
    out_d = nc.dram_tensor("partials", [PP, 5], f32, kind="ExternalOutput")

    AF = mybir.ActivationFunctionType
    ALU = mybir.AluOpType

    with (
        nc.sbuf_tensor([PP, 2 * FF], f32) as xt,
        nc.sbuf_tensor([PP, 1], f32) as warm,
        nc.sbuf_tensor([PP, FF], f32) as p,
        nc.sbuf_tensor([PP, FF], f32) as tcopy,
        nc.sbuf_tensor([PP, FF], f32) as p2,
        nc.sbuf_tensor([PP, FF], f32) as pt,
        nc.sbuf_tensor([PP, FF], f32) as p2t,
        nc.sbuf_tensor([PP, 5], f32) as acc,
        nc.semaphore("dma_in") as dma_in,
        nc.semaphore("dma_in_g") as dma_in_g,
        nc.semaphore("act_done") as act_done,
        nc.semaphore("dve_done") as dve_done,
        nc.Block() as block,
    ):
        xa = xt[:, 0:FF]
        tf = xt[:, FF : 2 * FF]
        const0 = nc.const_aps.tensor(0.0, (PP, 1), f32)

        dma_engine = "gpsimd" if variant == "v2g" else "sync"

        in_sem = dma_in_g if dma_engine == "gpsimd" else dma_in

        def dma_prog(eng):
            eng.dma_start(
                xt[:], xt_d[:], single_packet=(variant in ("v2sp", "v5"))
            ).then_inc(in_sem, 16)

        if dma_engine == "sync":

            @block.sync
            def _(sync):
                dma_prog(sync)
                sync.wait_ge(act_done, 2)
                sync.wait_ge(dve_done, 3)
                sync.dma_start(out_d[:], acc[:]).then_inc(dma_in, 16)
        else:

            @block.gpsimd
            def _(gpsimd):
                dma_prog(gpsimd)

            @block.sync
            def _(sync):
                sync.wait_ge(act_done, 2)
                sync.wait_ge(dve_done, 3)
                sync.dma_start(out_d[:], acc[:]).then_inc(dma_in, 16)

        @block.scalar
        def _(scalar):
            # Prime the Sigmoid PWP table before the data arrives.
            scalar.activation(warm[:], const0, AF.Sigmoid)
            scalar.wait_ge(in_sem, 16)
            # p = sigmoid(x); acc[:,0] = rowsum(p)
            scalar.activation(
                p[:], xa, AF.Sigmoid, accum_out=acc[:, 0:1]
            ).then_inc(act_done, 1)
            # acc[:,4] = rowsum(t)
            scalar.activation(
                tcopy[:], tf, AF.Copy, accum_out=acc[:, 4:5]
            ).then_inc(act_done, 1)

        @block.vector
        def _(vector):
            vector.wait_ge(act_done, 1)
            # p2 = (p*1)*p; acc[:,1] = rowsum(p2)
            vector.scalar_tensor_tensor(
                out=p2[:], in0=p[:], scalar=1.0, in1=p[:],
                op0=ALU.mult, op1=ALU.mult, accum_out=acc[:, 1:2],
            ).then_inc(dve_done, 1)
            # pt = (p*1)*t; acc[:,2] = rowsum(pt)
            vector.scalar_tensor_tensor(
                out=pt[:], in0=p[:], scalar=1.0, in1=tf,
                op0=ALU.mult, op1=ALU.mult, accum_out=acc[:, 2:3],
            ).then_inc(dve_done, 1)
            # p2t = (p2*1)*t; acc[:,3] = rowsum(p2t) — wait for the p2 write
            # to retire (same-engine RAW is not interlocked)
            vector.wait_ge(dve_done, 1)
            vector.scalar_tensor_tensor(
                out=p2t[:], in0=p2[:], scalar=1.0, in1=tf,
                op0=ALU.mult, op1=ALU.mult, accum_out=acc[:, 3:4],
            ).then_inc(dve_done, 1)

    return nc


def _build_bass_v4(nc, bass, mybir):
    """Split inputs: 4KB x-DMA on sync (gates the sigmoid), t-DMA on gpsimd
    in parallel; output DMA issued by the scalar engine itself."""
    f32 = mybir.dt.float32
    AF = mybir.ActivationFunctionType
    ALU = mybir.AluOpType

    x_d = nc.dram_tensor("x", [P, F], f32, kind="ExternalInput")
    t_d = nc.dram_tensor("t", [P, F], f32, kind="ExternalInput")
    out_d = nc.dram_tensor("partials", [P, 5], f32, kind="ExternalOutput")

    with (
        nc.sbuf_tensor([P, F], f32) as xa,
        nc.sbuf_tensor([P, F], f32) as tf,
        nc.sbuf_tensor([P, 1], f32) as warm,
        nc.sbuf_tensor([P, F], f32) as p,
        nc.sbuf_tensor([P, F], f32) as tcopy,
        nc.sbuf_tensor([P, F], f32) as p2,
        nc.sbuf_tensor([P, F], f32) as pt,
        nc.sbuf_tensor([P, F], f32) as p2t,
        nc.sbuf_tensor([P, 5], f32) as acc,
        nc.semaphore("dma_x") as dma_x,
        nc.semaphore("dma_t") as dma_t,
        nc.semaphore("dma_out_sem") as dma_out_sem,
        nc.semaphore("act_done") as act_done,
        nc.semaphore("dve_done") as dve_done,
        nc.Block() as block,
    ):
        const0 = nc.const_aps.tensor(0.0, (P, 1), f32)

        @block.sync
        def _(sync):
            sync.dma_start(xa[:], x_d[:], single_packet=True).then_inc(dma_x, 16)

        @block.gpsimd
        def _(gpsimd):
            gpsimd.dma_start(tf[:], t_d[:]).then_inc(dma_t, 16)

        @block.scalar
        def _(scalar):
            # Prime the Sigmoid PWP table before the data arrives.
            scalar.activation(warm[:], const0, AF.Sigmoid)
            scalar.wait_ge(dma_x, 16)
            scalar.activation(
                p[:], xa[:], AF.Sigmoid, accum_out=acc[:, 0:1]
            ).then_inc(act_done, 1)
            scalar.wait_ge(dma_t, 16)
            scalar.activation(
                tcopy[:], tf[:], AF.Copy, accum_out=acc[:, 4:5]
            ).then_inc(act_done, 1)
            scalar.wait_ge(act_done, 2)
            scalar.wait_ge(dve_done, 3)
            scalar.dma_start(out_d[:], acc[:]).then_inc(dma_out_sem, 16)

        @block.vector
        def _(vector):
            vector.wait_ge(act_done, 1)
            vector.scalar_tensor_tensor(
                out=p2[:], in0=p[:], scalar=1.0, in1=p[:],
                op0=ALU.mult, op1=ALU.mult, accum_out=acc[:, 1:2],
            ).then_inc(dve_done, 1)
            vector.wait_ge(dma_t, 16)
            vector.scalar_tensor_tensor(
                out=pt[:], in0=p[:], scalar=1.0, in1=tf[:],
                op0=ALU.mult, op1=ALU.mult, accum_out=acc[:, 2:3],
            ).then_inc(dve_done, 1)
            vector.wait_ge(dve_done, 1)
            vector.scalar_tensor_tensor(
                out=p2t[:], in0=p2[:], scalar=1.0, in1=tf[:],
                op0=ALU.mult, op1=ALU.mult, accum_out=acc[:, 3:4],
            ).then_inc(dve_done, 1)

    return nc


def _build_floor():
    """Minimal kernel: one tiny output DMA — measures the NEFF protocol floor."""
    import concourse.bass as bass
    import concourse.mybir as mybir

    nc = bass.Bass()
    f32 = mybir.dt.float32
    out_d = nc.dram_tensor("partials", [P, 1], f32, kind="ExternalOutput")
    with nc.Block() as block:
        const0 = nc.const_aps.tensor(0.0, (P, 1), f32)

        @block.sync
        def _(sync):
            with nc.semaphore("floor_sem") as fs:
                sync.dma_start(out_d[:], const0).then_inc(fs, 16)

    return nc


def _get_nc():
    global _NC
    if _NC is None:
        _NC = _build_bass(VARIANT)
    return _NC


def _make_in_maps_v4(y_pred, y_true):
    x = np.asarray(y_pred, dtype=np.float32).reshape(-1)
    t = np.asarray(y_true).astype(np.float32).reshape(-1)
    return [
        {
            "x": np.ascontiguousarray(x[c * SHARD : (c + 1) * SHARD].reshape(P, F)),
            "t": np.ascontiguousarray(t[c * SHARD : (c + 1) * SHARD].reshape(P, F)),
        }
        for c in range(N_CORES)
    ]


def _make_in_maps(y_pred, y_true, pp=None):
    pp = VP if pp is None else pp
    ff = SHARD // pp
    x = np.asarray(y_pred, dtype=np.float32).reshape(-1)
    t = np.asarray(y_true).astype(np.float32).reshape(-1)
    in_maps = []
    for c in range(N_CORES):
        sl = slice(c * SHARD, (c + 1) * SHARD)
        xt = np.concatenate(
            [x[sl].reshape(pp, ff), t[sl].reshape(pp, ff)], axis=1
        )
        in_maps.append({"xt": np.ascontiguousarray(xt)})
    return in_maps


def _combine(partials_list):
    # partials_list: per-core [P, 5] float32 arrays
    S = np.zeros(5, dtype=np.float64)
    for part in partials_list:
        S += part.astype(np.float64).sum(axis=0)
    S1, S2, Spt, Sp2t, St = S
    n = float(N)
    n_pos = St
    n_neg = n - St
    sum_dist_sq = 2.0 * n * S2 - 2.0 * S1 * S1
    ss_pos = Sp2t - Spt * Spt / n_pos
    Sn = S1 - Spt
    Sn2 = S2 - Sp2t
    ss_neg = Sn2 - Sn * Sn / n_neg
    loss = (
        sum_dist_sq * (2.0 * n_pos * n_neg) / (n * n)
        + (ss_pos + ss_neg) * (n_pos * n_pos + n_neg * n_neg) / (n * n)
    )
    return np.asarray(loss, dtype=np.float32)


def kernel(y_pred, y_true, epoch=None, **_unused):
    from concourse.bass_utils import run_bass_kernel_spmd

    nc = _get_nc()
    in_maps = _make_in_maps(y_pred, y_true)
    res = run_bass_kernel_spmd(nc, in_maps, list(range(N_CORES)))
    partials = [r["partials"] for r in res.results]
    return _combine(partials)


# revision 22
# speedup vs baseline: 1.2948x; 1.0040x over previous
"""Contrastive-loss kernel for Trainium2 (8 NeuronCores, SPMD).

The reference builds NxN pairwise matrices, but every term collapses to a
closed form over five O(N) reductions of p = sigmoid(y_pred) and t = y_true:

    S1 = sum p          S2 = sum p^2
    Spt = sum p*t       Sp2t = sum p^2*t      St = sum t

    sum_dist_sq = 2*N*S2 - 2*S1^2
    mean(loss_diff) = sum_dist_sq * 2*n_pos*n_neg / N^2
    ss_pos + ss_neg = (Sp2t - Spt^2/n_pos) + ((S2-Sp2t) - (S1-Spt)^2/n_neg)
    mean(loss_same) = (ss_pos+ss_neg) * (n_pos^2+n_neg^2) / N^2

Each of the 8 cores reduces a 1024-element shard (x and t packed as one
[32, 64] tile so the input lands in a single DMA; 32 partitions measured
marginally faster than 128 — shorter output DMA and accumulator reads) and
emits [32, 5] per-partition partials; the host sums partials in float64 and
applies the closed form.

Device-side structure per core (raw Bass, manual semaphores):
  sync  : DMA xt in -> (wait compute) -> DMA partials out (completion is
          covered by the block-exit DRAIN, no extra sem round-trip)
  scalar: prime Sigmoid PWP table on a const AP before the DMA wait (the
          ~1.3us table load overlaps the input DMA), then
          Sigmoid(x)+rowsum(p), Copy(t)+rowsum(t)
  vector: three scalar_tensor_tensor ops with fused row-sum accumulators:
          p^2, p*t, p^2*t
"""

import numpy as np

N = 8192
N_CORES = 8
SHARD = N // N_CORES  # 1024
P = 128
F = SHARD // P  # 8

VARIANT = "v5"  # [32, 64] tiles, single-packet input DMA
VP = 32         # partitions used by the default variant
VF = SHARD // VP

_NC = None  # compiled Bass program, built once


def _build_bass(variant="v2"):
    import concourse.bass as bass
    import concourse.mybir as mybir

    nc = bass.Bass()
    f32 = mybir.dt.float32

    if variant == "v4":
        return _build_bass_v4(nc, bass, mybir)

    # v5: same structure as v2sp but [32, 64] tiles — fewer partitions means
    # fewer DMA descriptor rows and shorter accumulator reads.
    # v6: v5 + output DMA issued by the scalar engine, so sync's preamble
    # (the entry-barrier straggler) carries only one DMA descriptor.
    PP = 32 if variant in ("v5", "v6") else P
    FF = SHARD // PP

    xt_d = nc.dram_tensor("xt", [PP, 2 * FF], f32, kind="ExternalInput")
    out_d = nc.dram_tensor("partials", [PP, 5], f32, kind="ExternalOutput")

    AF = mybir.ActivationFunctionType
    ALU = mybir.AluOpType

    with (
        nc.sbuf_tensor([PP, 2 * FF], f32) as xt,
        nc.sbuf_tensor([PP, 1], f32) as warm,
        nc.sbuf_tensor([PP, FF], f32) as p,
        nc.sbuf_tensor([PP, FF], f32) as tcopy,
        nc.sbuf_tensor([PP, FF], f32) as p2,
        nc.sbuf_tensor([PP, FF], f32) as pt,
        nc.sbuf_tensor([PP, FF], f32) as p2t,
        nc.sbuf_tensor([PP, 5], f32) as acc,
        nc.semaphore("dma_in") as dma_in,
        nc.semaphore("dma_in_g") as dma_in_g,
        nc.semaphore("act_done") as act_done,
        nc.semaphore("dve_done") as dve_done,
        nc.Block() as block,
    ):
        xa = xt[:, 0:FF]
        tf = xt[:, FF : 2 * FF]
        const0 = nc.const_aps.tensor(0.0, (PP, 1), f32)

        dma_engine = "gpsimd" if variant == "v2g" else "sync"

        in_sem = dma_in_g if dma_engine == "gpsimd" else dma_in

        def dma_prog(eng):
            eng.dma_start(
                xt[:], xt_d[:], single_packet=(variant in ("v2sp", "v5", "v6"))
            ).then_inc(in_sem, 16)

        if variant == "v6":

            @block.sync
            def _(sync):
                dma_prog(sync)
        elif dma_engine == "sync":

            @block.sync
            def _(sync):
                dma_prog(sync)
                sync.wait_ge(act_done, 2)
                sync.wait_ge(dve_done, 3)
                sync.dma_start(out_d[:], acc[:]).then_inc(dma_in, 16)
        else:

            @block.gpsimd
            def _(gpsimd):
                dma_prog(gpsimd)

            @block.sync
            def _(sync):
                sync.wait_ge(act_done, 2)
                sync.wait_ge(dve_done, 3)
                sync.dma_start(out_d[:], acc[:]).then_inc(dma_in, 16)

        @block.scalar
        def _(scalar):
            # Prime the Sigmoid PWP table before the data arrives.
            scalar.activation(warm[:], const0, AF.Sigmoid)
            scalar.wait_ge(in_sem, 16)
            # p = sigmoid(x); acc[:,0] = rowsum(p)
            scalar.activation(
                p[:], xa, AF.Sigmoid, accum_out=acc[:, 0:1]
            ).then_inc(act_done, 1)
            # acc[:,4] = rowsum(t)
            scalar.activation(
                tcopy[:], tf, AF.Copy, accum_out=acc[:, 4:5]
            ).then_inc(act_done, 1)
            if variant == "v6":
                # own Copy's accum write must retire before the DMA reads acc
                scalar.wait_ge(act_done, 2)
                scalar.wait_ge(dve_done, 3)
                scalar.dma_start(out_d[:], acc[:]).then_inc(dma_in_g, 16)

        @block.vector
        def _(vector):
            vector.wait_ge(act_done, 1)
            # p2 = (p*1)*p; acc[:,1] = rowsum(p2)
            vector.scalar_tensor_tensor(
                out=p2[:], in0=p[:], scalar=1.0, in1=p[:],
                op0=ALU.mult, op1=ALU.mult, accum_out=acc[:, 1:2],
            ).then_inc(dve_done, 1)
            # pt = (p*1)*t; acc[:,2] = rowsum(pt)
            vector.scalar_tensor_tensor(
                out=pt[:], in0=p[:], scalar=1.0, in1=tf,
                op0=ALU.mult, op1=ALU.mult, accum_out=acc[:, 2:3],
            ).then_inc(dve_done, 1)
            # p2t = (p2*1)*t; acc[:,3] = rowsum(p2t) — wait for the p2 write
            # to retire (same-engine RAW is not interlocked)
            vector.wait_ge(dve_done, 1)
            vector.scalar_tensor_tensor(
                out=p2t[:], in0=p2[:], scalar=1.0, in1=tf,
                op0=ALU.mult, op1=ALU.mult, accum_out=acc[:, 3:4],
            ).then_inc(dve_done, 1)

    return nc


def _build_bass_v4(nc, bass, mybir):
    """Split inputs: 4KB x-DMA on sync (gates the sigmoid), t-DMA on gpsimd
    in parallel; output DMA issued by the scalar engine itself."""
    f32 = mybir.dt.float32
    AF = mybir.ActivationFunctionType
    ALU = mybir.AluOpType

    x_d = nc.dram_tensor("x", [P, F], f32, kind="ExternalInput")
    t_d = nc.dram_tensor("t", [P, F], f32, kind="ExternalInput")
    out_d = nc.dram_tensor("partials", [P, 5], f32, kind="ExternalOutput")

    with (
        nc.sbuf_tensor([P, F], f32) as xa,
        nc.sbuf_tensor([P, F], f32) as tf,
        nc.sbuf_tensor([P, 1], f32) as warm,
        nc.sbuf_tensor([P, F], f32) as p,
        nc.sbuf_tensor([P, F], f32) as tcopy,
        nc.sbuf_tensor([P, F], f32) as p2,
        nc.sbuf_tensor([P, F], f32) as pt,
        nc.sbuf_tensor([P, F], f32) as p2t,
        nc.sbuf_tensor([P, 5], f32) as acc,
        nc.semaphore("dma_x") as dma_x,
        nc.semaphore("dma_t") as dma_t,
        nc.semaphore("dma_out_sem") as dma_out_sem,
        nc.semaphore("act_done") as act_done,
        nc.semaphore("dve_done") as dve_done,
        nc.Block() as block,
    ):
        const0 = nc.const_aps.tensor(0.0, (P, 1), f32)

        @block.sync
        def _(sync):
            sync.dma_start(xa[:], x_d[:], single_packet=True).then_inc(dma_x, 16)

        @block.gpsimd
        def _(gpsimd):
            gpsimd.dma_start(tf[:], t_d[:]).then_inc(dma_t, 16)

        @block.scalar
        def _(scalar):
            # Prime the Sigmoid PWP table before the data arrives.
            scalar.activation(warm[:], const0, AF.Sigmoid)
            scalar.wait_ge(dma_x, 16)
            scalar.activation(
                p[:], xa[:], AF.Sigmoid, accum_out=acc[:, 0:1]
            ).then_inc(act_done, 1)
            scalar.wait_ge(dma_t, 16)
            scalar.activation(
                tcopy[:], tf[:], AF.Copy, accum_out=acc[:, 4:5]
            ).then_inc(act_done, 1)
            scalar.wait_ge(act_done, 2)
            scalar.wait_ge(dve_done, 3)
            scalar.dma_start(out_d[:], acc[:]).then_inc(dma_out_sem, 16)

        @block.vector
        def _(vector):
            vector.wait_ge(act_done, 1)
            vector.scalar_tensor_tensor(
                out=p2[:], in0=p[:], scalar=1.0, in1=p[:],
                op0=ALU.mult, op1=ALU.mult, accum_out=acc[:, 1:2],
            ).then_inc(dve_done, 1)
            vector.wait_ge(dma_t, 16)
            vector.scalar_tensor_tensor(
                out=pt[:], in0=p[:], scalar=1.0, in1=tf[:],
                op0=ALU.mult, op1=ALU.mult, accum_out=acc[:, 2:3],
            ).then_inc(dve_done, 1)
            vector.wait_ge(dve_done, 1)
            vector.scalar_tensor_tensor(
                out=p2t[:], in0=p2[:], scalar=1.0, in1=tf[:],
                op0=ALU.mult, op1=ALU.mult, accum_out=acc[:, 3:4],
            ).then_inc(dve_done, 1)

    return nc


def _build_floor():
    """Minimal kernel: one tiny output DMA — measures the NEFF protocol floor."""
    import concourse.bass as bass
    import concourse.mybir as mybir

    nc = bass.Bass()
    f32 = mybir.dt.float32
    out_d = nc.dram_tensor("partials", [P, 1], f32, kind="ExternalOutput")
    with nc.Block() as block:
        const0 = nc.const_aps.tensor(0.0, (P, 1), f32)

        @block.sync
        def _(sync):
            with nc.semaphore("floor_sem") as fs:
                sync.dma_start(out_d[:], const0).then_inc(fs, 16)

    return nc


def _get_nc():
    global _NC
    if _NC is None:
        _NC = _build_bass(VARIANT)
    return _NC


def _make_in_maps_v4(y_pred, y_true):
    x = np.asarray(y_pred, dtype=np.float32).reshape(-1)
    t = np.asarray(y_true).astype(np.float32).reshape(-1)
    return [
        {
            "x": np.ascontiguousarray(x[c * SHARD : (c + 1) * SHARD].reshape(P, F)),
            "t": np.ascontiguousarray(t[c * SHARD : (c + 1) * SHARD].reshape(P, F)),
        }
        for c in range(N_CORES)
    ]


def _make_in_maps(y_pred, y_true, pp=None):
    pp = VP if pp is None else pp
    ff = SHARD // pp
    x = np.asarray(y_pred, dtype=np.float32).reshape(-1)
    t = np.asarray(y_true).astype(np.float32).reshape(-1)
    in_maps = []
    for c in range(N_CORES):
        sl = slice(c * SHARD, (c + 1) * SHARD)
        xt = np.concatenate(
            [x[sl].reshape(pp, ff), t[sl].reshape(pp, ff)], axis=1
        )
        in_maps.append({"xt": np.ascontiguousarray(xt)})
    return in_maps


def _combine(partials_list):
    # partials_list: per-core [P, 5] float32 arrays
    S = np.zeros(5, dtype=np.float64)
    for part in partials_list:
        S += part.astype(np.float64).sum(axis=0)
    S1, S2, Spt, Sp2t, St = S
    n = float(N)
    n_pos = St
    n_neg = n - St
    sum_dist_sq = 2.0 * n * S2 - 2.0 * S1 * S1
    ss_pos = Sp2t - Spt * Spt / n_pos
    Sn = S1 - Spt
    Sn2 = S2 - Sp2t
    ss_neg = Sn2 - Sn * Sn / n_neg
    loss = (
        sum_dist_sq * (2.0 * n_pos * n_neg) / (n * n)
        + (ss_pos + ss_neg) * (n_pos * n_pos + n_neg * n_neg) / (n * n)
    )
    return np.asarray(loss, dtype=np.float32)


def kernel(y_pred, y_true, epoch=None, **_unused):
    from concourse.bass_utils import run_bass_kernel_spmd

    nc = _get_nc()
    in_maps = _make_in_maps(y_pred, y_true)
    res = run_bass_kernel_spmd(nc, in_maps, list(range(N_CORES)))
    partials = [r["partials"] for r in res.results]
    return _combine(partials)
